# revision 92
# baseline (speedup 1.0000x reference)
"""BiPhaseScorer Trainium2 kernel (8 NeuronCores, SPMD). v2

Sharding: core (b, g) = batch b in {0,1} x head-group g in {0..3} (2 heads each).
Each core: projects its batch's tokens onto its 2 heads' QKV slices, runs
bi-phase attention (phase + magnitude scores, softmax), applies its heads'
slice of the output projections, and writes per-core partial outputs
[S, E] for x and y channels. Host sums partials over head-groups + bias.

Math: cos(arctan2(y,x)) = x/r, sin = y/r with r = sqrt(x^2+y^2), so
  scores = BETA*(cosq.cosk + sinq.sink)/D + (1-BETA)*(rq.rk)/sqrt(D)
is three rank-D contractions; cos/sin stack to one K=128 matmul. All matmul
scaling is folded into the Q-side host-prep (weights scaled by ALPHA) and the
on-device sqrt (scale=1/C1^2), so scores come out of PSUM fully scaled.
Softmax skips max-subtraction (scores bounded ~O(10) for this regime; exp is
safe in fp32).

v2 engine-balance changes vs v1 (201us -> 162us in the cost model):
  - sqrt via ln+exp on ACT (u=ln(sc*t+eps); rc=exp(-.5u); r=(t*sc)*rc on DVE)
    so every ACT func (Square/Ln/Exp) lives in one act-table set; the
    remaining redundant LoadActFuncSet insts are merged post-finalize.
  - cos/sin via fused scalar_tensor_tensor (bias add folded), no standalone
    bias adds; the r^2 add runs on DVE (latency-critical A chain).
  - es in bf16; softmax denominator via bf16 pair-adds (DVE) + quad-adds
    (Pool) + gpsimd partition_all_reduce (which also broadcasts), replacing
    the 16-deep f32 accumulation chain and the PE broadcast matmul.
  - pa is drained to SBUF right after its matmuls so the psA rotation
    buffer frees ~5us earlier (the x-side chain reads the copy); the sin
    stt runs first so pb's buffer frees early too.
  - sn partition-shift DMAs issue on the SP queue (Pool SWDGE holds blocked
    the quad-adds; ACT holds blocked the exp stream).
  - out-proj (phase C) interleaved per-qt; rounds run K->Q->V with round 0
    as K->V->Q to match DMA arrival order (in-order PE queue).
  - startup DMAs ordered so the first projection's deps land first.
  - K(0) pipelined in 256-column halves (earlier phase-B chunk release);
    every block's denominator is a DVE running sum so only one add trails
    its final exp — the short chain releases the po banks the next qt's
    attnV waits on, and shortens the kernel tail.
  - the two head-blocks of each qt interleave at chunk granularity: two
    independent score->exp->attnV streams absorb per-chunk jitter (PE and
    ACT run near-lockstep at ~639 vs ~612 ns/chunk), with deep es/pair
    buffer rotations to decouple them; the tail qt's normalization muls
    are column-halved so out-proj starts one half-mul earlier.
  - Q/K activations stream in fp8e4m3 (halves 16 of 24 input tiles'
    bandwidth in the DMA-bound early phase). V stays bf16: this attention
    is highly peaked (mag-dominated scores), so V quantization error does
    NOT average out across keys (fp8 V alone measured 2.4e-2). QK-fp8
    lands at 7.6e-3 vs the 2e-2 gate.  201us -> 149us overall.

Layouts (per core):
  cs_q[h]/cs_k[h] [128, S] f32r: h0 = [cos(0:64); sin(64:128)], h1 = [sin; cos]
  r_q/r_k         [128, S] f32r: heads packed [h0 | h1] on partitions
  v_sb            [128, KC, 256] bf16: per key-chunk, free = per-head 128 cols,
                  h0 = [vx|vy], h1 = [vy|vx] (swap lets O^T land lane-local
                  in the x/y-stacked ot tensors below)
  otx             [128, S] f32r: [h0 Ox^T (0:64); h1 Ox^T (64:128)]
  oty             [128, S] f32r: [h1 Oy^T (0:64); h0 Oy^T (64:128)]
Out-proj is then a single Kc=128 matmul per (token block, channel).
"""

import numpy as np
import ml_dtypes

import concourse.bacc as bacc
import concourse.mybir as mybir
from concourse.tile import TileContext
from concourse import bass_isa
from concourse.bass_utils import run_bass_kernel_spmd

B, S, E, H = 2, 2048, 512, 8
D = E // H              # 64
BETA = 0.5
SCALE = float(np.sqrt(D))
C1 = BETA / D
C2 = (1.0 - BETA) / SCALE
ALPHA = C1 * C2
SQRT_SCALE = (C2 / ALPHA) ** 2  # == 1/C1^2

NCORES = 8
HG = 2
D2 = HG * D             # 128
EC = E // 128           # 4
TT = 4                  # 512-token tiles
QT = 4
KC = S // 128           # 16
TB = S // 128           # 16

F32 = mybir.dt.float32
F32R = mybir.dt.float32r
BF16 = mybir.dt.bfloat16

TRACE = False
LAST_RESULTS = None

ADD = mybir.AluOpType.add
MULT = mybir.AluOpType.mult
AF = mybir.ActivationFunctionType


def build_bass(stage="full"):
    nc = bacc.Bacc("TRN2", target_bir_lowering=False, debug=False,
                   enable_asserts=True, num_devices=NCORES)

    xs = {}
    F8 = mybir.dt.float8e4
    for n in ["xqx", "xqy", "xkx", "xky"]:
        xs[n] = nc.dram_tensor(n, [E, S], F8, kind="ExternalInput")
    for n in ["xvx", "xvy"]:
        xs[n] = nc.dram_tensor(n, [E, S], BF16, kind="ExternalInput")
    ws = {}
    for n in ["wqx", "wqy", "wkx", "wky", "wvx", "wvy"]:
        ws[n] = nc.dram_tensor(n, [E, D2], BF16, kind="ExternalInput")
    # x/y-stacked output projections [128, E] (see module docstring)
    wox = nc.dram_tensor("wox", [D2, E], F32R, kind="ExternalInput")
    woy = nc.dram_tensor("woy", [D2, E], F32R, kind="ExternalInput")
    bs = {n: nc.dram_tensor(n, [D2], F32, kind="ExternalInput")
          for n in ["bqx", "bqy", "bkx", "bky"]}
    yx = nc.dram_tensor("yx", [S, E], F32, kind="ExternalOutput")
    yy = nc.dram_tensor("yy", [S, E], F32, kind="ExternalOutput")

    with TileContext(nc) as tc:
        with (
            tc.tile_pool(name="persist", bufs=1) as pp,
            tc.tile_pool(name="wpool", bufs=1) as wp,
            tc.tile_pool(name="stream", bufs=4) as sp,
            tc.tile_pool(name="tmp", bufs=2) as tp,
            tc.tile_pool(name="psA", bufs=1, space="PSUM") as psA,
            tc.tile_pool(name="psB", bufs=1, space="PSUM") as psB,
        ):
            cs_q = [[pp.tile([128, 512], F32R, tag=f"cs_q{h}_{t}", name=f"cs_q{h}_{t}")
                     for t in range(QT)] for h in range(HG)]
            cs_k = [[pp.tile([128, 512], F32R, tag=f"cs_k{h}_{t}", name=f"cs_k{h}_{t}")
                     for t in range(TT)] for h in range(HG)]
            r_q = [pp.tile([128, 512], F32R, tag=f"r_q{t}", name=f"r_q{t}") for t in range(QT)]
            r_k = [pp.tile([128, 512], F32R, tag=f"r_k{t}", name=f"r_k{t}") for t in range(TT)]
            v_sb = [pp.tile([128, 2 * D2], BF16, tag=f"v_sb{t}", name=f"v_sb{t}")
                    for t in range(KC)]
            otx = [pp.tile([128, 512], F32R, tag=f"otx{t}", name=f"otx{t}") for t in range(QT)]
            oty = [pp.tile([128, 512], F32R, tag=f"oty{t}", name=f"oty{t}") for t in range(QT)]

            w_sb = {n: wp.tile([128, EC, D2], BF16, tag=f"w_{n}", name=f"w_{n}")
                    for n in ws}
            wox_sb = wp.tile([D2, E], F32R, tag="wox")
            woy_sb = wp.tile([D2, E], F32R, tag="woy")
            b_sb = {n: wp.tile([D2, 1], F32, tag=f"b_{n}", name=f"b_{n}") for n in bs}

            def load_w(names):
                for n in names:
                    nc.sync.dma_start(w_sb[n][:], ws[n].ap().rearrange(
                        "(c p) d -> p c d", p=128))
            def load_b(names):
                for n in names:
                    nc.sync.dma_start(b_sb[n][:], bs[n].ap().unsqueeze(1))

            eps_sb = wp.tile([128, 1], F32, tag="eps")
            nc.vector.memset(eps_sb[:], 1e-20)

            def load_x(name, tt):
                tsl = slice(tt * 512, (tt + 1) * 512)
                dt_ = xs[name].dtype
                xt = sp.tile([128, EC, 512], dt_, tag=f"xt_{str(dt_)}", bufs=4,
                             name=f"{name}_{tt}")
                nc.sync.dma_start(xt[:], xs[name].ap().rearrange(
                    "(c p) t -> p c t", p=128)[:, :, tsl])
                return xt

            def qk_side(xa, xb, nwx, nwy, nbx, nby, cs_t, r_t, sc, tt,
                        split=1):
                """split=2 pipelines the side in 256-column halves so the
                first half of cs/r is usable ~3.5us earlier (K sides feed
                phase-B chunk availability via column slices)."""
                pa_ps = psA.tile([128, 512], F32, tag="proj", bufs=3, name="pa")
                pb = psA.tile([128, 512], F32, tag="proj", bufs=3, name="pb")
                W = 512 // split
                for hf in range(split):
                    sl = slice(hf * W, (hf + 1) * W)
                    for ec in range(EC):
                        nc.tensor.matmul(pa_ps[:, sl], w_sb[nwx][:, ec],
                                         xa[:, ec, sl],
                                         start=(ec == 0), stop=(ec == EC - 1))
                    # drain pa to SBUF immediately: the x-side chain reads
                    # the copy, so this rotation buffer frees sooner and the
                    # next side's projections aren't blocked on the ACT chain
                    pa_h = tp.tile([128, W], F32, tag="pacp", bufs=4, name="pa_sb")
                    nc.vector.tensor_copy(pa_h[:], pa_ps[:, sl])
                    for ec in range(EC):
                        nc.tensor.matmul(pb[:, sl], w_sb[nwy][:, ec],
                                         xb[:, ec, sl],
                                         start=(ec == 0), stop=(ec == EC - 1))
                    t0 = tp.tile([128, W], F32, tag="t0", name="t0")
                    t1 = tp.tile([128, W], F32, tag="t1", name="t1")
                    nc.scalar.activation(t0[:], pa_h[:], AF.Square,
                                         bias=b_sb[nbx][:])
                    nc.scalar.activation(t1[:], pb[:, sl], AF.Square,
                                         bias=b_sb[nby][:])
                    nc.vector.tensor_add(t0[:], t0[:], t1[:])
                    # r = sqrt(sc*t0) via ln/exp: ACT stays on one table set
                    lu = tp.tile([128, W], F32, tag="lu", name="lu")
                    nc.scalar.activation(lu[:], t0[:], AF.Ln,
                                         bias=eps_sb[:], scale=float(sc))
                    rc = tp.tile([128, W], F32, tag="rc", name="rc")
                    nc.scalar.activation(rc[:], lu[:], AF.Exp, scale=-0.5)
                    cs0 = cs_t[0][tt]
                    cs1 = cs_t[1][tt]
                    with nc.allow_low_precision(reason="f32r rounding"):
                        # sin first: it is pb's last reader, so the psA
                        # buffer frees as early as possible
                        sn = tp.tile([128, W], F32R, tag="sn", name="sn")
                        nc.vector.scalar_tensor_tensor(
                            sn[:], pb[:, sl], b_sb[nby][:], rc[:],
                            op0=ADD, op1=MULT)
                        nc.vector.scalar_tensor_tensor(
                            r_t[tt][:, sl], t0[:], float(sc), rc[:],
                            op0=MULT, op1=MULT)
                        # cos halves straight into stacks (lane-local)
                        nc.vector.scalar_tensor_tensor(
                            cs0[0:64, sl], pa_h[0:64], b_sb[nbx][0:64],
                            rc[0:64], op0=ADD, op1=MULT)
                        nc.vector.scalar_tensor_tensor(
                            cs1[64:128, sl], pa_h[64:128], b_sb[nbx][64:128],
                            rc[64:128], op0=ADD, op1=MULT)
                    nc.sync.dma_start(cs0[64:128, sl], sn[0:64])
                    nc.sync.dma_start(cs1[0:64, sl], sn[64:128])

            def v_round(xvx_t, xvy_t, tt):
                for sub in range(4):
                    tb = tt * 4 + sub
                    ssl = slice(sub * 128, (sub + 1) * 128)
                    pv = psA.tile([128, 1024], F32, tag="proj", bufs=1,
                                  name="pv")[:, 0:2 * D2]
                    for ec in range(EC):
                        nc.tensor.matmul(pv[:, 0:D2], xvx_t[:, ec, ssl],
                                         w_sb["wvx"][:, ec],
                                         start=(ec == 0), stop=False)
                    for ec in range(EC):
                        nc.tensor.matmul(pv[:, D2:2 * D2], xvy_t[:, ec, ssl],
                                         w_sb["wvy"][:, ec],
                                         start=(ec == 0), stop=(ec == EC - 1))
                    # psum input-major [vx_h0|vx_h1|vy_h0|vy_h1] ->
                    # v_sb h0 = [vx_h0|vy_h0], h1 = [vy_h1|vx_h1]
                    vt = v_sb[tb][:].rearrange("p (i z) -> p i z", i=4)
                    pvv = pv[:].rearrange("p (i z) -> p i z", i=4)
                    with nc.allow_low_precision(reason="bf16 V"):
                        nc.vector.tensor_copy(vt[:, 0::2], pvv[:, 0::3])
                        nc.vector.tensor_copy(vt[:, 1::2], pvv[:, 2:0:-1])

            # phase A: interleaved rounds K(tt) -> V(tt) -> Q(tt), so phase B
            # can start on early kc chunks while later tiles still project.
            # Round 0 input DMAs are issued before the remaining weights so
            # the first projection starts ~2us in.
            # first projection's deps stream first; biases are consumed
            # only at the Square, so they come after the x tiles
            load_w(["wkx"])
            xk0a = load_x("xkx", 0)
            load_w(["wky"])
            xk0 = (xk0a, load_x("xky", 0))
            load_b(["bkx", "bky"])
            load_w(["wvx", "wvy"])
            xv0 = (load_x("xvx", 0), load_x("xvy", 0))
            load_w(["wqx", "wqy"])
            load_b(["bqx", "bqy"])
            xq0 = (load_x("xqx", 0), load_x("xqy", 0))
            nc.sync.dma_start(wox_sb[:], wox.ap())
            nc.sync.dma_start(woy_sb[:], woy.ap())

            for tt in range(TT):
                xa, xb = xk0 if tt == 0 else (load_x("xkx", tt), load_x("xky", tt))
                qk_side(xa, xb, "wkx", "wky", "bkx", "bky", cs_k, r_k, 1.0, tt,
                        split=(2 if tt == 0 else 1))
                if tt == 0:
                    # round 0: V before Q — it matches DMA arrival order, so
                    # the in-order PE queue isn't blocked on the later xq0
                    v_round(xv0[0], xv0[1], 0)
                    qk_side(xq0[0], xq0[1], "wqx", "wqy", "bqx", "bqy",
                            cs_q, r_q, SQRT_SCALE, 0)
                else:
                    xa, xb = load_x("xqx", tt), load_x("xqy", tt)
                    qk_side(xa, xb, "wqx", "wqy", "bqx", "bqy", cs_q, r_q,
                            SQRT_SCALE, tt)
                    xvx_t, xvy_t = load_x("xvx", tt), load_x("xvy", tt)
                    v_round(xvx_t, xvy_t, tt)

            def out_proj(qt):
                for sub in range(4):
                    tb = qt * 4 + sub
                    tsl = slice(tb * 128, (tb + 1) * 128)
                    sb_ = sub * 128
                    pxy = psA.tile([128, 1024], F32, tag="proj", bufs=1,
                                    name="pxy")
                    nc.tensor.matmul(pxy[:, 0:E], otx[qt][:, sb_:sb_ + 128],
                                     wox_sb[:], start=True, stop=True)
                    nc.tensor.matmul(pxy[:, E:2 * E], oty[qt][:, sb_:sb_ + 128],
                                     woy_sb[:], start=True, stop=True)
                    oxy = tp.tile([128, 2 * E], F32, tag="oxy", name="oxy")
                    nc.vector.tensor_copy(oxy[:], pxy[:])
                    nc.sync.dma_start(yx.ap()[tsl, :], oxy[:, 0:E])
                    nc.sync.dma_start(yy.ap()[tsl, :], oxy[:, E:2 * E])

            # phase B: per (qt, h) block over 16 key chunks; es in bf16.
            # attnV is software-pipelined one chunk behind the exp. The
            # softmax denominator: pair-adds on DVE, quad-adds on Pool, then
            # 4 PSUM-accumulated ones-matmuls per block.
            for qt in range(QT):
                # the two head-blocks of a qt interleave at chunk
                # granularity: two independent score->exp->attnV streams
                # keep PE and ACT fed through per-chunk jitter, and each
                # head's denominator tail hides behind the other's compute
                last_qt = (qt == QT - 1)
                po = [psB.tile([128, 512], F32, tag="o", bufs=2,
                               name=f"po{h}") for h in range(HG)]
                es_l = [[], []]
                pair = [None, None]
                quads = [[], []]
                # in the last qt, h0 leads h1 by LEAD chunks so h0's
                # denominator/mul tail overlaps h1's remaining compute and
                # only h1's chain is exposed at the kernel tail
                LEAD = 0
                h0_done = [False]
                for kci in range(KC + LEAD):
                    if kci == KC and LEAD:
                        finish_h(0)
                        h0_done[0] = True
                    for h in range(HG):
                        kc = kci if h == 0 else kci - LEAD
                        if not (0 <= kc < KC):
                            continue
                        hsl = slice(h * 64, (h + 1) * 64)
                        kt, ko = kc // 4, (kc % 4) * 128
                        ps = psB.tile([128, 512], F32, tag="s", bufs=3, name="ps")
                        nc.tensor.matmul(ps[:], cs_k[h][kt][:, ko:ko + 128],
                                         cs_q[h][qt][:, :],
                                         start=True, stop=False)
                        nc.tensor.matmul(ps[:], r_k[kt][hsl, ko:ko + 128],
                                         r_q[qt][hsl, :],
                                         start=False, stop=True)
                        es = tp.tile([128, 512], BF16, tag="es", bufs=8, name="es")
                        with nc.allow_low_precision(reason="bf16 attn weights"):
                            nc.scalar.activation(es[:], ps[:], AF.Exp)
                        es_l[h].append(es)
                        if kc > 0:
                            pv_ = v_sb[kc - 1][:, h * D2:(h + 1) * D2]
                            nc.tensor.matmul(po[h][:], pv_, es_l[h][kc - 1][:],
                                             start=(kc == 1), stop=False)
                        with nc.allow_low_precision(reason="bf16 denominator"):
                            if kc % 2 == 1:
                                p = tp.tile([128, 512], BF16, tag="pair",
                                            bufs=4, name="pair")
                                nc.vector.tensor_add(p[:], es_l[h][kc - 1][:],
                                                     es[:])
                                if last_qt:
                                    # tail qt: running sums so only one add
                                    # trails each head's final exp
                                    r2 = tp.tile([128, 512], BF16, tag="run",
                                                 bufs=4, name="run")
                                    if not quads[h]:
                                        nc.vector.tensor_copy(r2[:], p[:])
                                    else:
                                        nc.vector.tensor_add(
                                            r2[:], quads[h][-1][:], p[:])
                                    quads[h].append(r2)
                                elif pair[h] is None:
                                    pair[h] = p
                                else:
                                    q = tp.tile([128, 512], BF16, tag="quad",
                                                bufs=8, name="quad")
                                    nc.gpsimd.tensor_add(q[:], pair[h][:], p[:])
                                    quads[h].append(q)
                                    pair[h] = None
                def finish_h(h):
                    nc.tensor.matmul(po[h][:],
                                     v_sb[KC - 1][:, h * D2:(h + 1) * D2],
                                     es_l[h][KC - 1][:], start=False, stop=True)
                    # finish the chunk-sum tree on DVE, then an all-partition
                    # reduce on Pool gives the broadcast denominator directly
                    if last_qt:
                        qq = quads[h][-1]
                    else:
                      with nc.allow_low_precision(reason="bf16 denominator"):
                        q01 = tp.tile([128, 512], BF16, tag="q01", bufs=2, name="q01")
                        q23 = tp.tile([128, 512], BF16, tag="q01", bufs=2, name="q23")
                        nc.vector.tensor_add(q01[:], quads[h][0][:], quads[h][1][:])
                        nc.vector.tensor_add(q23[:], quads[h][2][:], quads[h][3][:])
                        qq = tp.tile([128, 512], BF16, tag="qq", bufs=2, name="qq")
                        nc.vector.tensor_add(qq[:], q01[:], q23[:])
                    den_bc = tp.tile([128, 512], F32, tag="denb", bufs=2, name="den_bc")
                    nc.gpsimd.partition_all_reduce(den_bc[:], qq[:], channels=128,
                                                   reduce_op=bass_isa.ReduceOp.add)
                    bc_sb = tp.tile([128, 512], F32, tag="bcs", name="bc_sb")
                    with nc.allow_low_precision(reason="softmax denom reciprocal"):
                        nc.vector.reciprocal(bc_sb[:], den_bc[:])
                    # h0: po = [Ox^T; Oy^T] -> otx[0:64], oty[64:128]
                    # h1: po = [Oy^T; Ox^T] -> oty[0:64], otx[64:128]
                    lo, hi = (otx[qt], oty[qt]) if h == 0 else (oty[qt], otx[qt])
                    if last_qt:
                        # tail: column-halved muls let the first out-proj
                        # matmuls start one half-mul earlier
                        for cs_ in (slice(0, 256), slice(256, 512)):
                            nc.vector.tensor_mul(lo[0:64, cs_],
                                                 po[h][0:64, cs_],
                                                 bc_sb[0:64, cs_])
                            nc.vector.tensor_mul(hi[64:128, cs_],
                                                 po[h][64:128, cs_],
                                                 bc_sb[64:128, cs_])
                    else:
                        nc.vector.tensor_mul(lo[0:64, :], po[h][0:64], bc_sb[0:64])
                        nc.vector.tensor_mul(hi[64:128, :], po[h][64:128],
                                             bc_sb[64:128])
                if not h0_done[0]:
                    finish_h(0)
                finish_h(1)
                # out-proj for this qt right away so the tail is short
                out_proj(qt)

    nc.finalize()
    _merge_act_table_loads(nc)
    return nc


def _merge_act_table_loads(nc):
    """All ACT funcs this kernel uses (Square/Ln/Exp) live together in act
    table set 6 (natural_log_exp_and_others), but the insertion pass assigns
    each func its first-matching set (square/exp->0, ln->5) and thrashes
    12+ reloads. Retarget every load to set 6 and drop the now-redundant
    ones (only those carrying no semaphore waits/updates)."""
    for blk in nc.m.functions[0].blocks:
        seen = False
        keep = []
        for inst in blk.instructions:
            if isinstance(inst, mybir.InstLoadActFuncSet):
                si = inst.sync_info
                has_sync = si is not None and (
                    len(si.on_wait) > 0 or len(si.on_update) > 0)
                if seen and not has_sync:
                    continue  # redundant reload, safe to drop
                inst.act_func_set_id = 6
                seen = True
            keep.append(inst)
        blk.instructions[:] = keep


_NC_CACHE = None


def make_in_maps(acts, W, bias):
    """acts: dict qx..vy [B,S,E] f32; W: dict Wqx..Woy; bias: dict bqx..boy."""
    f32 = np.float32
    bf16 = ml_dtypes.bfloat16
    in_maps = []
    for core in range(NCORES):
        b, g = core // 4, core % 4
        gs = slice(g * D2, (g + 1) * D2)
        h0 = slice((2 * g) * D, (2 * g + 1) * D)
        h1 = slice((2 * g + 1) * D, (2 * g + 2) * D)
        m = {}
        f8 = ml_dtypes.float8_e4m3
        m["xqx"] = np.ascontiguousarray(acts["qx"][b].T).astype(f8)
        m["xqy"] = np.ascontiguousarray(acts["qy"][b].T).astype(f8)
        m["xkx"] = np.ascontiguousarray(acts["kx"][b].T).astype(f8)
        m["xky"] = np.ascontiguousarray(acts["ky"][b].T).astype(f8)
        m["xvx"] = np.ascontiguousarray(acts["vx"][b].T).astype(bf16)
        m["xvy"] = np.ascontiguousarray(acts["vy"][b].T).astype(bf16)
        m["wqx"] = (np.ascontiguousarray(W["Wqx"][gs].T) * f32(ALPHA)).astype(bf16)
        m["wqy"] = (np.ascontiguousarray(W["Wqy"][gs].T) * f32(ALPHA)).astype(bf16)
        m["wkx"] = np.ascontiguousarray(W["Wkx"][gs].T).astype(bf16)
        m["wky"] = np.ascontiguousarray(W["Wky"][gs].T).astype(bf16)
        m["wvx"] = np.ascontiguousarray(W["Wvx"][gs].T).astype(bf16)
        m["wvy"] = np.ascontiguousarray(W["Wvy"][gs].T).astype(bf16)
        # otx partitions = (h0 dx, h1 dx); oty partitions = (h1 dy, h0 dy)
        m["wox"] = np.ascontiguousarray(
            np.concatenate([W["Wox"][:, h0].T, W["Wox"][:, h1].T], axis=0))
        m["woy"] = np.ascontiguousarray(
            np.concatenate([W["Woy"][:, h1].T, W["Woy"][:, h0].T], axis=0))
        m["bqx"] = bias["bqx"][gs] * f32(ALPHA)
        m["bqy"] = bias["bqy"][gs] * f32(ALPHA)
        m["bkx"] = np.ascontiguousarray(bias["bkx"][gs])
        m["bky"] = np.ascontiguousarray(bias["bky"][gs])
        in_maps.append(m)
    return in_maps


def kernel(qx, qy, kx, ky, vx, vy,
           Wqx, bqx, Wqy, bqy, Wkx, bkx, Wky, bky,
           Wvx, bvx, Wvy, bvy, Wox, box, Woy, boy):
    global _NC_CACHE, LAST_RESULTS
    f32 = np.float32
    acts = {"qx": qx, "qy": qy, "kx": kx, "ky": ky, "vx": vx, "vy": vy}
    acts = {k: np.asarray(v, f32) for k, v in acts.items()}
    W = {"Wqx": Wqx, "Wqy": Wqy, "Wkx": Wkx, "Wky": Wky,
         "Wvx": Wvx, "Wvy": Wvy, "Wox": Wox, "Woy": Woy}
    W = {k: np.asarray(v, f32) for k, v in W.items()}
    bias = {"bqx": bqx, "bqy": bqy, "bkx": bkx, "bky": bky,
            "bvx": bvx, "bvy": bvy}
    bias = {k: np.asarray(v, f32) for k, v in bias.items()}
    box, boy = np.asarray(box, f32), np.asarray(boy, f32)

    if _NC_CACHE is None:
        _NC_CACHE = build_bass()
    nc = _NC_CACHE

    in_maps = make_in_maps(acts, W, bias)
    # device execution can flake (NRT_EXEC_UNIT_UNRECOVERABLE observed once
    # on an otherwise-identical program) -> retry once before giving up
    try:
        res = run_bass_kernel_spmd(nc, in_maps, core_ids=list(range(NCORES)),
                                   trace=TRACE)
    except Exception:
        import time
        time.sleep(5)
        res = run_bass_kernel_spmd(nc, in_maps, core_ids=list(range(NCORES)),
                                   trace=TRACE)
    LAST_RESULTS = res

    out_x = np.zeros((B, S, E), f32)
    out_y = np.zeros((B, S, E), f32)
    for core in range(NCORES):
        b = core // 4
        out_x[b] += res.results[core]["yx"]
        out_y[b] += res.results[core]["yy"]
    out_x += box + bias["bvx"] @ W["Wox"].T
    out_y += boy + bias["bvy"] @ W["Woy"].T
    return out_x, out_y


# revision 93
# speedup vs baseline: 1.0048x; 1.0048x over previous
"""BiPhaseScorer Trainium2 kernel (8 NeuronCores, SPMD). v2

Sharding: core (b, g) = batch b in {0,1} x head-group g in {0..3} (2 heads each).
Each core: projects its batch's tokens onto its 2 heads' QKV slices, runs
bi-phase attention (phase + magnitude scores, softmax), applies its heads'
slice of the output projections, and writes per-core partial outputs
[S, E] for x and y channels. Host sums partials over head-groups + bias.

Math: cos(arctan2(y,x)) = x/r, sin = y/r with r = sqrt(x^2+y^2), so
  scores = BETA*(cosq.cosk + sinq.sink)/D + (1-BETA)*(rq.rk)/sqrt(D)
is three rank-D contractions; cos/sin stack to one K=128 matmul. All matmul
scaling is folded into the Q-side host-prep (weights scaled by ALPHA) and the
on-device sqrt (scale=1/C1^2), so scores come out of PSUM fully scaled.
Softmax skips max-subtraction (scores bounded ~O(10) for this regime; exp is
safe in fp32).

v2 engine-balance changes vs v1 (201us -> 162us in the cost model):
  - sqrt via ln+exp on ACT (u=ln(sc*t+eps); rc=exp(-.5u); r=(t*sc)*rc on DVE)
    so every ACT func (Square/Ln/Exp) lives in one act-table set; the
    remaining redundant LoadActFuncSet insts are merged post-finalize.
  - cos/sin via fused scalar_tensor_tensor (bias add folded), no standalone
    bias adds; the r^2 add runs on DVE (latency-critical A chain).
  - es in bf16; softmax denominator via bf16 pair-adds (DVE) + quad-adds
    (Pool) + gpsimd partition_all_reduce (which also broadcasts), replacing
    the 16-deep f32 accumulation chain and the PE broadcast matmul.
  - pa is drained to SBUF right after its matmuls so the psA rotation
    buffer frees ~5us earlier (the x-side chain reads the copy); the sin
    stt runs first so pb's buffer frees early too.
  - sn partition-shift DMAs issue on the SP queue (Pool SWDGE holds blocked
    the quad-adds; ACT holds blocked the exp stream).
  - out-proj (phase C) interleaved per-qt; rounds run K->Q->V with round 0
    as K->V->Q to match DMA arrival order (in-order PE queue).
  - startup DMAs ordered so the first projection's deps land first.
  - K(0) pipelined in 256-column halves (earlier phase-B chunk release);
    every block's denominator is a DVE running sum so only one add trails
    its final exp — the short chain releases the po banks the next qt's
    attnV waits on, and shortens the kernel tail.
  - the two head-blocks of each qt interleave at chunk granularity: two
    independent score->exp->attnV streams absorb per-chunk jitter (PE and
    ACT run near-lockstep at ~639 vs ~612 ns/chunk), with deep es/pair
    buffer rotations to decouple them; the tail qt's normalization muls
    are column-halved so out-proj starts one half-mul earlier.
  - Q/K activations stream in fp8e4m3 (halves 16 of 24 input tiles'
    bandwidth in the DMA-bound early phase). V stays bf16: this attention
    is highly peaked (mag-dominated scores), so V quantization error does
    NOT average out across keys (fp8 V alone measured 2.4e-2). QK-fp8
    lands at 7.6e-3 vs the 2e-2 gate.  201us -> 149us overall.

Layouts (per core):
  cs_q[h]/cs_k[h] [128, S] f32r: h0 = [cos(0:64); sin(64:128)], h1 = [sin; cos]
  r_q/r_k         [128, S] f32r: heads packed [h0 | h1] on partitions
  v_sb            [128, KC, 256] bf16: per key-chunk, free = per-head 128 cols,
                  h0 = [vx|vy], h1 = [vy|vx] (swap lets O^T land lane-local
                  in the x/y-stacked ot tensors below)
  otx             [128, S] f32r: [h0 Ox^T (0:64); h1 Ox^T (64:128)]
  oty             [128, S] f32r: [h1 Oy^T (0:64); h0 Oy^T (64:128)]
Out-proj is then a single Kc=128 matmul per (token block, channel).
"""

import numpy as np
import ml_dtypes

import concourse.bacc as bacc
import concourse.mybir as mybir
from concourse.tile import TileContext
from concourse import bass_isa
from concourse.bass_utils import run_bass_kernel_spmd

B, S, E, H = 2, 2048, 512, 8
D = E // H              # 64
BETA = 0.5
SCALE = float(np.sqrt(D))
C1 = BETA / D
C2 = (1.0 - BETA) / SCALE
ALPHA = C1 * C2
SQRT_SCALE = (C2 / ALPHA) ** 2  # == 1/C1^2

NCORES = 8
HG = 2
D2 = HG * D             # 128
EC = E // 128           # 4
TT = 4                  # 512-token tiles
QT = 4
KC = S // 128           # 16
TB = S // 128           # 16

F32 = mybir.dt.float32
F32R = mybir.dt.float32r
BF16 = mybir.dt.bfloat16

TRACE = False
LAST_RESULTS = None

ADD = mybir.AluOpType.add
MULT = mybir.AluOpType.mult
AF = mybir.ActivationFunctionType


def build_bass(stage="full"):
    nc = bacc.Bacc("TRN2", target_bir_lowering=False, debug=False,
                   enable_asserts=True, num_devices=NCORES)

    xs = {}
    F8 = mybir.dt.float8e4
    for n in ["xqx", "xqy", "xkx", "xky"]:
        xs[n] = nc.dram_tensor(n, [E, S], F8, kind="ExternalInput")
    for n in ["xvx", "xvy"]:
        xs[n] = nc.dram_tensor(n, [E, S], BF16, kind="ExternalInput")
    ws = {}
    for n in ["wqx", "wqy", "wkx", "wky", "wvx", "wvy"]:
        ws[n] = nc.dram_tensor(n, [E, D2], BF16, kind="ExternalInput")
    # x/y-stacked output projections [128, E] (see module docstring)
    wox = nc.dram_tensor("wox", [D2, E], F32R, kind="ExternalInput")
    woy = nc.dram_tensor("woy", [D2, E], F32R, kind="ExternalInput")
    bs = {n: nc.dram_tensor(n, [D2], F32, kind="ExternalInput")
          for n in ["bqx", "bqy", "bkx", "bky"]}
    yx = nc.dram_tensor("yx", [S, E], F32, kind="ExternalOutput")
    yy = nc.dram_tensor("yy", [S, E], F32, kind="ExternalOutput")

    with TileContext(nc) as tc:
        with (
            tc.tile_pool(name="persist", bufs=1) as pp,
            tc.tile_pool(name="wpool", bufs=1) as wp,
            tc.tile_pool(name="stream", bufs=4) as sp,
            tc.tile_pool(name="tmp", bufs=2) as tp,
            tc.tile_pool(name="psA", bufs=1, space="PSUM") as psA,
            tc.tile_pool(name="psB", bufs=1, space="PSUM") as psB,
        ):
            cs_q = [[pp.tile([128, 512], F32R, tag=f"cs_q{h}_{t}", name=f"cs_q{h}_{t}")
                     for t in range(QT)] for h in range(HG)]
            cs_k = [[pp.tile([128, 512], F32R, tag=f"cs_k{h}_{t}", name=f"cs_k{h}_{t}")
                     for t in range(TT)] for h in range(HG)]
            r_q = [pp.tile([128, 512], F32R, tag=f"r_q{t}", name=f"r_q{t}") for t in range(QT)]
            r_k = [pp.tile([128, 512], F32R, tag=f"r_k{t}", name=f"r_k{t}") for t in range(TT)]
            v_sb = [pp.tile([128, 2 * D2], BF16, tag=f"v_sb{t}", name=f"v_sb{t}")
                    for t in range(KC)]
            otx = [pp.tile([128, 512], F32R, tag=f"otx{t}", name=f"otx{t}") for t in range(QT)]
            oty = [pp.tile([128, 512], F32R, tag=f"oty{t}", name=f"oty{t}") for t in range(QT)]

            w_sb = {n: wp.tile([128, EC, D2], BF16, tag=f"w_{n}", name=f"w_{n}")
                    for n in ws}
            wox_sb = wp.tile([D2, E], F32R, tag="wox")
            woy_sb = wp.tile([D2, E], F32R, tag="woy")
            b_sb = {n: wp.tile([D2, 1], F32, tag=f"b_{n}", name=f"b_{n}") for n in bs}

            def load_w(names):
                for n in names:
                    nc.sync.dma_start(w_sb[n][:], ws[n].ap().rearrange(
                        "(c p) d -> p c d", p=128))
            def load_b(names):
                for n in names:
                    nc.sync.dma_start(b_sb[n][:], bs[n].ap().unsqueeze(1))

            eps_sb = wp.tile([128, 1], F32, tag="eps")
            nc.vector.memset(eps_sb[:], 1e-20)

            def load_x(name, tt):
                tsl = slice(tt * 512, (tt + 1) * 512)
                dt_ = xs[name].dtype
                xt = sp.tile([128, EC, 512], dt_, tag=f"xt_{str(dt_)}", bufs=4,
                             name=f"{name}_{tt}")
                nc.sync.dma_start(xt[:], xs[name].ap().rearrange(
                    "(c p) t -> p c t", p=128)[:, :, tsl])
                return xt

            def qk_side(xa, xb, nwx, nwy, nbx, nby, cs_t, r_t, sc, tt,
                        split=1):
                """split=2 pipelines the side in 256-column halves so the
                first half of cs/r is usable ~3.5us earlier (K sides feed
                phase-B chunk availability via column slices)."""
                pa_ps = psA.tile([128, 512], F32, tag="proj", bufs=3, name="pa")
                pb = psA.tile([128, 512], F32, tag="proj", bufs=3, name="pb")
                W = 512 // split
                for hf in range(split):
                    sl = slice(hf * W, (hf + 1) * W)
                    for ec in range(EC):
                        nc.tensor.matmul(pa_ps[:, sl], w_sb[nwx][:, ec],
                                         xa[:, ec, sl],
                                         start=(ec == 0), stop=(ec == EC - 1))
                    # drain pa to SBUF immediately: the x-side chain reads
                    # the copy, so this rotation buffer frees sooner and the
                    # next side's projections aren't blocked on the ACT chain
                    pa_h = tp.tile([128, W], F32, tag="pacp", bufs=4, name="pa_sb")
                    nc.vector.tensor_copy(pa_h[:], pa_ps[:, sl])
                    for ec in range(EC):
                        nc.tensor.matmul(pb[:, sl], w_sb[nwy][:, ec],
                                         xb[:, ec, sl],
                                         start=(ec == 0), stop=(ec == EC - 1))
                    t0 = tp.tile([128, W], F32, tag="t0", name="t0")
                    t1 = tp.tile([128, W], F32, tag="t1", name="t1")
                    nc.scalar.activation(t0[:], pa_h[:], AF.Square,
                                         bias=b_sb[nbx][:])
                    nc.scalar.activation(t1[:], pb[:, sl], AF.Square,
                                         bias=b_sb[nby][:])
                    nc.vector.tensor_add(t0[:], t0[:], t1[:])
                    # r = sqrt(sc*t0) via ln/exp: ACT stays on one table set
                    lu = tp.tile([128, W], F32, tag="lu", name="lu")
                    nc.scalar.activation(lu[:], t0[:], AF.Ln,
                                         bias=eps_sb[:], scale=float(sc))
                    rc = tp.tile([128, W], F32, tag="rc", name="rc")
                    nc.scalar.activation(rc[:], lu[:], AF.Exp, scale=-0.5)
                    cs0 = cs_t[0][tt]
                    cs1 = cs_t[1][tt]
                    with nc.allow_low_precision(reason="f32r rounding"):
                        # sin first: it is pb's last reader, so the psA
                        # buffer frees as early as possible
                        sn = tp.tile([128, W], F32R, tag="sn", name="sn")
                        nc.vector.scalar_tensor_tensor(
                            sn[:], pb[:, sl], b_sb[nby][:], rc[:],
                            op0=ADD, op1=MULT)
                        nc.vector.scalar_tensor_tensor(
                            r_t[tt][:, sl], t0[:], float(sc), rc[:],
                            op0=MULT, op1=MULT)
                        # cos halves straight into stacks (lane-local)
                        nc.vector.scalar_tensor_tensor(
                            cs0[0:64, sl], pa_h[0:64], b_sb[nbx][0:64],
                            rc[0:64], op0=ADD, op1=MULT)
                        nc.vector.scalar_tensor_tensor(
                            cs1[64:128, sl], pa_h[64:128], b_sb[nbx][64:128],
                            rc[64:128], op0=ADD, op1=MULT)
                    nc.sync.dma_start(cs0[64:128, sl], sn[0:64])
                    nc.sync.dma_start(cs1[0:64, sl], sn[64:128])

            def v_round(xvx_t, xvy_t, tt):
                for sub in range(4):
                    tb = tt * 4 + sub
                    ssl = slice(sub * 128, (sub + 1) * 128)
                    pv = psA.tile([128, 1024], F32, tag="proj", bufs=1,
                                  name="pv")[:, 0:2 * D2]
                    for ec in range(EC):
                        nc.tensor.matmul(pv[:, 0:D2], xvx_t[:, ec, ssl],
                                         w_sb["wvx"][:, ec],
                                         start=(ec == 0), stop=False)
                    for ec in range(EC):
                        nc.tensor.matmul(pv[:, D2:2 * D2], xvy_t[:, ec, ssl],
                                         w_sb["wvy"][:, ec],
                                         start=(ec == 0), stop=(ec == EC - 1))
                    # psum input-major [vx_h0|vx_h1|vy_h0|vy_h1] ->
                    # v_sb h0 = [vx_h0|vy_h0], h1 = [vy_h1|vx_h1]
                    vt = v_sb[tb][:].rearrange("p (i z) -> p i z", i=4)
                    pvv = pv[:].rearrange("p (i z) -> p i z", i=4)
                    with nc.allow_low_precision(reason="bf16 V"):
                        nc.vector.tensor_copy(vt[:, 0::2], pvv[:, 0::3])
                        nc.vector.tensor_copy(vt[:, 1::2], pvv[:, 2:0:-1])

            # phase A: interleaved rounds K(tt) -> V(tt) -> Q(tt), so phase B
            # can start on early kc chunks while later tiles still project.
            # Round 0 input DMAs are issued before the remaining weights so
            # the first projection starts ~2us in.
            # first projection's deps stream first; biases are consumed
            # only at the Square, so they come after the x tiles
            load_w(["wkx"])
            xk0a = load_x("xkx", 0)
            load_w(["wky"])
            xk0 = (xk0a, load_x("xky", 0))
            load_b(["bkx", "bky"])
            load_w(["wvx", "wvy"])
            xv0 = (load_x("xvx", 0), load_x("xvy", 0))
            load_w(["wqx", "wqy"])
            load_b(["bqx", "bqy"])
            xq0 = (load_x("xqx", 0), load_x("xqy", 0))
            nc.sync.dma_start(wox_sb[:], wox.ap())
            nc.sync.dma_start(woy_sb[:], woy.ap())

            for tt in range(TT):
                xa, xb = xk0 if tt == 0 else (load_x("xkx", tt), load_x("xky", tt))
                qk_side(xa, xb, "wkx", "wky", "bkx", "bky", cs_k, r_k, 1.0, tt,
                        split=(2 if tt == 0 else 1))
                if tt == 0:
                    qk_side(xq0[0], xq0[1], "wqx", "wqy", "bqx", "bqy",
                            cs_q, r_q, SQRT_SCALE, 0)
                    v_round(xv0[0], xv0[1], 0)
                else:
                    xa, xb = load_x("xqx", tt), load_x("xqy", tt)
                    qk_side(xa, xb, "wqx", "wqy", "bqx", "bqy", cs_q, r_q,
                            SQRT_SCALE, tt)
                    xvx_t, xvy_t = load_x("xvx", tt), load_x("xvy", tt)
                    v_round(xvx_t, xvy_t, tt)

            def out_proj(qt):
                for sub in range(4):
                    tb = qt * 4 + sub
                    tsl = slice(tb * 128, (tb + 1) * 128)
                    sb_ = sub * 128
                    pxy = psA.tile([128, 1024], F32, tag="proj", bufs=1,
                                    name="pxy")
                    nc.tensor.matmul(pxy[:, 0:E], otx[qt][:, sb_:sb_ + 128],
                                     wox_sb[:], start=True, stop=True)
                    nc.tensor.matmul(pxy[:, E:2 * E], oty[qt][:, sb_:sb_ + 128],
                                     woy_sb[:], start=True, stop=True)
                    oxy = tp.tile([128, 2 * E], F32, tag="oxy", name="oxy")
                    nc.vector.tensor_copy(oxy[:], pxy[:])
                    nc.sync.dma_start(yx.ap()[tsl, :], oxy[:, 0:E])
                    nc.sync.dma_start(yy.ap()[tsl, :], oxy[:, E:2 * E])

            # phase B: per (qt, h) block over 16 key chunks; es in bf16.
            # attnV is software-pipelined one chunk behind the exp. The
            # softmax denominator: pair-adds on DVE, quad-adds on Pool, then
            # 4 PSUM-accumulated ones-matmuls per block.
            for qt in range(QT):
                # the two head-blocks of a qt interleave at chunk
                # granularity: two independent score->exp->attnV streams
                # keep PE and ACT fed through per-chunk jitter, and each
                # head's denominator tail hides behind the other's compute
                last_qt = (qt == QT - 1)
                po = [psB.tile([128, 512], F32, tag="o", bufs=2,
                               name=f"po{h}") for h in range(HG)]
                es_l = [[], []]
                pair = [None, None]
                quads = [[], []]
                # in the last qt, h0 leads h1 by LEAD chunks so h0's
                # denominator/mul tail overlaps h1's remaining compute and
                # only h1's chain is exposed at the kernel tail
                LEAD = 0
                h0_done = [False]
                for kci in range(KC + LEAD):
                    if kci == KC and LEAD:
                        finish_h(0)
                        h0_done[0] = True
                    for h in range(HG):
                        kc = kci if h == 0 else kci - LEAD
                        if not (0 <= kc < KC):
                            continue
                        hsl = slice(h * 64, (h + 1) * 64)
                        kt, ko = kc // 4, (kc % 4) * 128
                        ps = psB.tile([128, 512], F32, tag="s", bufs=3, name="ps")
                        nc.tensor.matmul(ps[:], cs_k[h][kt][:, ko:ko + 128],
                                         cs_q[h][qt][:, :],
                                         start=True, stop=False)
                        nc.tensor.matmul(ps[:], r_k[kt][hsl, ko:ko + 128],
                                         r_q[qt][hsl, :],
                                         start=False, stop=True)
                        es = tp.tile([128, 512], BF16, tag="es", bufs=8, name="es")
                        with nc.allow_low_precision(reason="bf16 attn weights"):
                            nc.scalar.activation(es[:], ps[:], AF.Exp)
                        es_l[h].append(es)
                        if kc > 0:
                            pv_ = v_sb[kc - 1][:, h * D2:(h + 1) * D2]
                            nc.tensor.matmul(po[h][:], pv_, es_l[h][kc - 1][:],
                                             start=(kc == 1), stop=False)
                        with nc.allow_low_precision(reason="bf16 denominator"):
                            if kc % 2 == 1:
                                p = tp.tile([128, 512], BF16, tag="pair",
                                            bufs=4, name="pair")
                                nc.vector.tensor_add(p[:], es_l[h][kc - 1][:],
                                                     es[:])
                                if last_qt:
                                    # tail qt: running sums so only one add
                                    # trails each head's final exp
                                    r2 = tp.tile([128, 512], BF16, tag="run",
                                                 bufs=4, name="run")
                                    if not quads[h]:
                                        nc.vector.tensor_copy(r2[:], p[:])
                                    else:
                                        nc.vector.tensor_add(
                                            r2[:], quads[h][-1][:], p[:])
                                    quads[h].append(r2)
                                elif pair[h] is None:
                                    pair[h] = p
                                else:
                                    q = tp.tile([128, 512], BF16, tag="quad",
                                                bufs=8, name="quad")
                                    nc.gpsimd.tensor_add(q[:], pair[h][:], p[:])
                                    quads[h].append(q)
                                    pair[h] = None
                def finish_h(h):
                    nc.tensor.matmul(po[h][:],
                                     v_sb[KC - 1][:, h * D2:(h + 1) * D2],
                                     es_l[h][KC - 1][:], start=False, stop=True)
                    # finish the chunk-sum tree on DVE, then an all-partition
                    # reduce on Pool gives the broadcast denominator directly
                    if last_qt:
                        qq = quads[h][-1]
                    else:
                      with nc.allow_low_precision(reason="bf16 denominator"):
                        q01 = tp.tile([128, 512], BF16, tag="q01", bufs=2, name="q01")
                        q23 = tp.tile([128, 512], BF16, tag="q01", bufs=2, name="q23")
                        nc.vector.tensor_add(q01[:], quads[h][0][:], quads[h][1][:])
                        nc.vector.tensor_add(q23[:], quads[h][2][:], quads[h][3][:])
                        qq = tp.tile([128, 512], BF16, tag="qq", bufs=2, name="qq")
                        nc.vector.tensor_add(qq[:], q01[:], q23[:])
                    den_bc = tp.tile([128, 512], F32, tag="denb", bufs=2, name="den_bc")
                    nc.gpsimd.partition_all_reduce(den_bc[:], qq[:], channels=128,
                                                   reduce_op=bass_isa.ReduceOp.add)
                    bc_sb = tp.tile([128, 512], F32, tag="bcs", name="bc_sb")
                    with nc.allow_low_precision(reason="softmax denom reciprocal"):
                        nc.vector.reciprocal(bc_sb[:], den_bc[:])
                    # h0: po = [Ox^T; Oy^T] -> otx[0:64], oty[64:128]
                    # h1: po = [Oy^T; Ox^T] -> oty[0:64], otx[64:128]
                    lo, hi = (otx[qt], oty[qt]) if h == 0 else (oty[qt], otx[qt])
                    if last_qt:
                        # tail: column-halved muls let the first out-proj
                        # matmuls start one half-mul earlier
                        for cs_ in (slice(0, 256), slice(256, 512)):
                            nc.vector.tensor_mul(lo[0:64, cs_],
                                                 po[h][0:64, cs_],
                                                 bc_sb[0:64, cs_])
                            nc.vector.tensor_mul(hi[64:128, cs_],
                                                 po[h][64:128, cs_],
                                                 bc_sb[64:128, cs_])
                    else:
                        nc.vector.tensor_mul(lo[0:64, :], po[h][0:64], bc_sb[0:64])
                        nc.vector.tensor_mul(hi[64:128, :], po[h][64:128],
                                             bc_sb[64:128])
                if not h0_done[0]:
                    finish_h(0)
                finish_h(1)
                # out-proj for this qt right away so the tail is short
                out_proj(qt)

    nc.finalize()
    _merge_act_table_loads(nc)
    return nc


def _merge_act_table_loads(nc):
    """All ACT funcs this kernel uses (Square/Ln/Exp) live together in act
    table set 6 (natural_log_exp_and_others), but the insertion pass assigns
    each func its first-matching set (square/exp->0, ln->5) and thrashes
    12+ reloads. Retarget every load to set 6 and drop the now-redundant
    ones (only those carrying no semaphore waits/updates)."""
    for blk in nc.m.functions[0].blocks:
        seen = False
        keep = []
        for inst in blk.instructions:
            if isinstance(inst, mybir.InstLoadActFuncSet):
                si = inst.sync_info
                has_sync = si is not None and (
                    len(si.on_wait) > 0 or len(si.on_update) > 0)
                if seen and not has_sync:
                    continue  # redundant reload, safe to drop
                inst.act_func_set_id = 6
                seen = True
            keep.append(inst)
        blk.instructions[:] = keep


_NC_CACHE = None


def make_in_maps(acts, W, bias):
    """acts: dict qx..vy [B,S,E] f32; W: dict Wqx..Woy; bias: dict bqx..boy."""
    f32 = np.float32
    bf16 = ml_dtypes.bfloat16
    in_maps = []
    for core in range(NCORES):
        b, g = core // 4, core % 4
        gs = slice(g * D2, (g + 1) * D2)
        h0 = slice((2 * g) * D, (2 * g + 1) * D)
        h1 = slice((2 * g + 1) * D, (2 * g + 2) * D)
        m = {}
        f8 = ml_dtypes.float8_e4m3
        m["xqx"] = np.ascontiguousarray(acts["qx"][b].T).astype(f8)
        m["xqy"] = np.ascontiguousarray(acts["qy"][b].T).astype(f8)
        m["xkx"] = np.ascontiguousarray(acts["kx"][b].T).astype(f8)
        m["xky"] = np.ascontiguousarray(acts["ky"][b].T).astype(f8)
        m["xvx"] = np.ascontiguousarray(acts["vx"][b].T).astype(bf16)
        m["xvy"] = np.ascontiguousarray(acts["vy"][b].T).astype(bf16)
        m["wqx"] = (np.ascontiguousarray(W["Wqx"][gs].T) * f32(ALPHA)).astype(bf16)
        m["wqy"] = (np.ascontiguousarray(W["Wqy"][gs].T) * f32(ALPHA)).astype(bf16)
        m["wkx"] = np.ascontiguousarray(W["Wkx"][gs].T).astype(bf16)
        m["wky"] = np.ascontiguousarray(W["Wky"][gs].T).astype(bf16)
        m["wvx"] = np.ascontiguousarray(W["Wvx"][gs].T).astype(bf16)
        m["wvy"] = np.ascontiguousarray(W["Wvy"][gs].T).astype(bf16)
        # otx partitions = (h0 dx, h1 dx); oty partitions = (h1 dy, h0 dy)
        m["wox"] = np.ascontiguousarray(
            np.concatenate([W["Wox"][:, h0].T, W["Wox"][:, h1].T], axis=0))
        m["woy"] = np.ascontiguousarray(
            np.concatenate([W["Woy"][:, h1].T, W["Woy"][:, h0].T], axis=0))
        m["bqx"] = bias["bqx"][gs] * f32(ALPHA)
        m["bqy"] = bias["bqy"][gs] * f32(ALPHA)
        m["bkx"] = np.ascontiguousarray(bias["bkx"][gs])
        m["bky"] = np.ascontiguousarray(bias["bky"][gs])
        in_maps.append(m)
    return in_maps


def kernel(qx, qy, kx, ky, vx, vy,
           Wqx, bqx, Wqy, bqy, Wkx, bkx, Wky, bky,
           Wvx, bvx, Wvy, bvy, Wox, box, Woy, boy):
    global _NC_CACHE, LAST_RESULTS
    f32 = np.float32
    acts = {"qx": qx, "qy": qy, "kx": kx, "ky": ky, "vx": vx, "vy": vy}
    acts = {k: np.asarray(v, f32) for k, v in acts.items()}
    W = {"Wqx": Wqx, "Wqy": Wqy, "Wkx": Wkx, "Wky": Wky,
         "Wvx": Wvx, "Wvy": Wvy, "Wox": Wox, "Woy": Woy}
    W = {k: np.asarray(v, f32) for k, v in W.items()}
    bias = {"bqx": bqx, "bqy": bqy, "bkx": bkx, "bky": bky,
            "bvx": bvx, "bvy": bvy}
    bias = {k: np.asarray(v, f32) for k, v in bias.items()}
    box, boy = np.asarray(box, f32), np.asarray(boy, f32)

    if _NC_CACHE is None:
        _NC_CACHE = build_bass()
    nc = _NC_CACHE

    in_maps = make_in_maps(acts, W, bias)
    # device execution can flake (NRT_EXEC_UNIT_UNRECOVERABLE observed once
    # on an otherwise-identical program) -> retry once before giving up
    try:
        res = run_bass_kernel_spmd(nc, in_maps, core_ids=list(range(NCORES)),
                                   trace=TRACE)
    except Exception:
        import time
        time.sleep(5)
        res = run_bass_kernel_spmd(nc, in_maps, core_ids=list(range(NCORES)),
                                   trace=TRACE)
    LAST_RESULTS = res

    out_x = np.zeros((B, S, E), f32)
    out_y = np.zeros((B, S, E), f32)
    for core in range(NCORES):
        b = core // 4
        out_x[b] += res.results[core]["yx"]
        out_y[b] += res.results[core]["yy"]
    out_x += box + bias["bvx"] @ W["Wox"].T
    out_y += boy + bias["bvy"] @ W["Woy"].T
    return out_x, out_y
